# revision 12
# baseline (speedup 1.0000x reference)
"""Multi-head attention (B=2, S=2048, D=1024, H=16, Dk=64) on 8 NeuronCores.

Sharding: 2-way data parallel over batch x 4-way tensor parallel over heads.
Core c = 4*b + g handles batch b, head group g (4 heads = 256 cols).
W_o is row-sliced; host sums the 4 partial outputs per batch (+ bv@Wo + bo).

v3 (elementwise-bound rework; ACT/DVE are the binding engines):
  - Q/K projections: fp8 DoubleRow as before, but the W-column permutation is
    chosen so each m-tile's psum [128,512] maps partition-straight into a
    single [128,2,S] qt8/kt8 tile (head h at partition base 32h, legal
    quadrant bases). The psum->sbuf move is ONE full-partition op per
    projection: DVE tensor_tensor add with a broadcast per-(p,m) bias tile;
    no /16 rescale (scales folded into the exp constant).
  - scores: per-head DR with dk=64 at quadrant base 32h (unchanged math).
  - softmax exp: ACT exact Exp and DVE Schraudolph split by weighted
    round-robin, scale = 0.125/256.
  - P@V FLIPPED: stationary = probs chunk [128,2,128], moving = vaug
    [128,2,65] (64 V-dims + 1 rowsum const col per head); out psum
    [128 tok, 4, 65]. Full A+R residual compensation (V stored as fp8 A
    plus equal-scale fp8 residual R). 65-wide outputs cut P@V PE time by
    ~35% and shrink the normalization to [128,4,64] broadcast ops.
  - normalization: DVE reciprocal [128,4] + one broadcast tensor_tensor mul
    writing bf16 attnP; DMA-transpose ([128,128] bf16 tiles, 14ns/xbar-tile)
    rebuilds the [dims, tok] layout for the output projection.
  - V path: V-proj psum -> ONE bf16 copy per qtr; Pool (gpsimd) builds fp8
    A and R from SBUF via tensor_tensor (GPSIMD cannot touch PSUM).
  - output projection: flipped (out^T [D,S] in DRAM, host transposes),
    moving = ot_t [128,2,512] bf16, stationary = Wo; psum->sbuf copies on
    ACT. bq/bk biases on-chip; bv/bo folded into the host-side reduction.
"""
import numpy as np
import ml_dtypes
from contextlib import ExitStack

import concourse.bass as bass
import concourse.mybir as mybir
import concourse.tile as tile
from concourse import bacc
from concourse.bass_utils import run_bass_kernel_spmd

F32 = mybir.dt.float32
F8 = mybir.dt.float8e4
I8 = mybir.dt.int8
BF = mybir.dt.bfloat16
E4 = ml_dtypes.float8_e4m3
BF16 = ml_dtypes.bfloat16
DR = mybir.MatmulPerfMode.DoubleRow

B, S, D = 2, 2048, 1024
P = 128
W = 256              # local width (4 heads x 64)
MT = 2               # m-tiles of 128
KO8 = D // P         # 8 bf16 contraction ktiles
KO4 = D // 256       # 4 DoubleRow ktiles
NQ = S // 512        # 4 sq chunks
NPAIR = S // 256     # 8 sk chunk-pairs
HW = 65              # per-head moving width in P@V (64 dims + rowsum col)
SCALE = 0.125 / 256.0          # qt8/kt8 carry x16 each
A_SCH = 8.0 / np.log(2.0) * SCALE
B_SCH = 55.6

_CACHE = {}


# exp engine split: deterministic weighted round-robin (Bresenham).
class _ExpAssign:
    def __init__(self, wa, wd):
        self.w = {"A": wa, "D": wd}
        self.acc = {"A": 0.0, "D": 0.0}

    def next(self):
        for k in self.acc:
            self.acc[k] += self.w[k]
        k = max(self.acc, key=lambda e: self.acc[e])
        self.acc[k] -= sum(self.w.values())
        return k


def build_nc():
    nc = bacc.Bacc("TRN2", target_bir_lowering=False, debug=False, num_devices=8)
    xb = nc.dram_tensor("xb", [D, S], BF, kind="ExternalInput").ap()
    x8 = nc.dram_tensor("x8", [D, S], F8, kind="ExternalInput").ap()
    wq8 = nc.dram_tensor("wq8", [D, W], F8, kind="ExternalInput").ap()
    wk8 = nc.dram_tensor("wk8", [D, W], F8, kind="ExternalInput").ap()
    wv = nc.dram_tensor("wv", [D, W], BF, kind="ExternalInput").ap()
    wo = nc.dram_tensor("wo", [W, D], BF, kind="ExternalInput").ap()
    bq = nc.dram_tensor("bq", [P, MT], F32, kind="ExternalInput").ap()
    bk = nc.dram_tensor("bk", [P, MT], F32, kind="ExternalInput").ap()
    out = nc.dram_tensor("out", [D, S], BF, kind="ExternalOutput").ap()

    x8_r = x8.rearrange("(ko t p) s -> p ko t s", p=P, t=2)
    xb_r = xb.rearrange("(ko p) s -> p ko s", p=P)
    wq8_r = wq8.rearrange("(ko t p) w -> p ko t w", p=P, t=2)
    wk8_r = wk8.rearrange("(ko t p) w -> p ko t w", p=P, t=2)
    wv_r = wv.rearrange("(ko p) w -> p ko w", p=P)
    wo_r = wo.rearrange("(m p) d -> p m d", p=P)

    with tile.TileContext(nc) as tc, ExitStack() as ctx:
        sb = ctx.enter_context(tc.tile_pool(name="sb", bufs=1))
        xp = ctx.enter_context(tc.tile_pool(name="xp", bufs=2))
        ptp = ctx.enter_context(tc.tile_pool(name="ptp", bufs=4))
        cp = ctx.enter_context(tc.tile_pool(name="cp", bufs=8))
        ps = ctx.enter_context(tc.tile_pool(name="ps", bufs=1, space="PSUM"))

        # ---- resident inputs (issue order = DMA priority) ----
        w8q_t = sb.tile([P, KO4, 2, W], F8)
        nc.scalar.dma_start(w8q_t[:], wq8_r)
        x8_tiles = [xp.tile([P, KO4, 2, 512], F8, tag="x8", name=f"x8_{i}",
                            bufs=4) for i in range(4)]

        def load_x8(qtr):
            sq = slice(qtr * 512, (qtr + 1) * 512)
            nc.sync.dma_start(x8_tiles[qtr][:, 0:2, :, :], x8_r[:, 0:2, :, sq])
            nc.sync.dma_start(x8_tiles[qtr][:, 2:4, :, :], x8_r[:, 2:4, :, sq])

        sq0 = slice(0, 512)
        nc.sync.dma_start(x8_tiles[0][:, 0:2, :, :], x8_r[:, 0:2, :, sq0])
        bq_t = sb.tile([P, MT], F32)
        nc.sync.dma_start(bq_t[:], bq)
        nc.sync.dma_start(x8_tiles[0][:, 2:4, :, :], x8_r[:, 2:4, :, sq0])
        w8k_t = sb.tile([P, KO4, 2, W], F8)
        nc.scalar.dma_start(w8k_t[:], wk8_r)
        bk_t = sb.tile([P, MT], F32)
        nc.sync.dma_start(bk_t[:], bk)

        xq_tiles = [None] * 4

        def load_xq(qtr):
            sq = slice(qtr * 512, (qtr + 1) * 512)
            xq_tiles[qtr] = xp.tile([P, KO8, 512], BF, tag="xq", bufs=2,
                                    name=f"xq{qtr}")
            nc.sync.dma_start(xq_tiles[qtr][:, 0:KO8 // 2, :],
                              xb_r[:, 0:KO8 // 2, sq])
            nc.sync.dma_start(xq_tiles[qtr][:, KO8 // 2:KO8, :],
                              xb_r[:, KO8 // 2:KO8, sq])

        load_xq(0)
        wv_t = sb.tile([P, KO8, W], BF)
        nc.sync.dma_start(wv_t[:], wv_r)
        for _q in (1, 2, 3):
            load_x8(_q)
        load_xq(1)
        wo_t = sb.tile([P, MT, D], BF)
        nc.sync.dma_start(wo_t[:], wo_r)

        zt = sb.tile([P, 1], BF)
        nc.vector.memset(zt[:], 0.0)

        # qt8/kt8: two [64, m, S] tiles (head-pairs); within a tile head
        # h%2 sits at base 32*(h%2) (base 96 is illegal for matmul operands),
        # dk = 32m + p%32
        qt8_m = [sb.tile([64, MT, S], F8, name=f"qt8_{j}") for j in range(2)]
        kt8_m = [sb.tile([64, MT, S], F8, name=f"kt8_{j}") for j in range(2)]
        # vaug A/R: [sk-part, pair, parity, head*65]; per head 64 V-dims
        # then 1 rowsum col (A: 16.0; R: 0.0)
        vaugA_t = sb.tile([P, NPAIR, 2, 4 * HW], F8)
        vaugR_t = sb.tile([P, NPAIR, 2, 4 * HW], F8)
        consA = vaugA_t[:].rearrange("p i t (h c) -> p i t h c", c=HW)[:, :, :, :, 64:HW]
        nc.gpsimd.memset(consA, 16.0)
        consR = vaugR_t[:].rearrange("p i t (h c) -> p i t h c", c=HW)[:, :, :, :, 64:HW]
        nc.gpsimd.memset(consR, 0.0)
        ot_t = sb.tile([P, MT, S], BF)

        ea_p1 = _ExpAssign(1.0, 1.0)      # phase-1 blocks
        ea_p2 = _ExpAssign(1.05, 1.0)     # phase-2 mix

        _uid = [0]
        pending_out = []
        pending_tp = []
        pending_dma = []

        ea = _ExpAssign(1.08, 1.0)

        attnP_tiles = {}

        def get_attnP(q):
            if q not in attnP_tiles:
                attnP_tiles[q] = cp.tile([P, 4, 2, P], BF, tag="attnP",
                                         bufs=2, name=f"attnP{q}")
            return attnP_tiles[q]

        def emit_tp(q, hp, c, attnP):
            nc.sync.dma_start(
                ot_t[:, hp, q * 512 + c * 128: q * 512 + (c + 1) * 128],
                attnP[:, c, hp, :], transpose=True)
            if hp == 1 and c == 3:
                outproj(q)

        def emit_dma(q, dc0, ob):
            sq = slice(q * 512, (q + 1) * 512)
            dr = out[dc0 * P:(dc0 + 2) * P, sq].rearrange(
                "(a p) n -> p a n", a=2)
            nc.sync.dma_start(dr, ob[:])

        _ob_cur = [None]

        def outproj_unit(q, dc, ceng):
            po = ps.tile([P, 512], F32, tag="po", bufs=1, name="po")
            sq = slice(q * 512, (q + 1) * 512)
            for m in range(MT):
                nc.tensor.matmul(po[:], wo_t[:, m, dc * P:(dc + 1) * P],
                                 ot_t[:, m, sq],
                                 start=(m == 0), stop=(m == MT - 1))
            if dc % 2 == 0:
                _ob_cur[0] = cp.tile([P, 2, 512], BF, tag="ob", bufs=3, name="ob")
            ob = _ob_cur[0]
            half = ob[:, dc % 2, :]
            if ceng == "A":
                nc.scalar.activation(half, po[:],
                                     mybir.ActivationFunctionType.Copy,
                                     bias=0.0, scale=1.0)
            else:
                nc.vector.tensor_copy(half, po[:])
            if dc % 2 == 1:
                pending_dma.append((q, dc - 1, ob))

        def outproj(q):
            for dc in range(8):
                pending_out.append((q, dc, "A" if dc % 4 != 3 else "D"))

        def qk_proj(qtr, wt, bt, dst):
            sq = slice(qtr * 512, (qtr + 1) * 512)
            ppt = ps.tile([P, 1024], F32, tag="sp", bufs=2, name="ppt")
            for m in range(MT):
                for ko in range(KO4):
                    nc.tensor.matmul(ppt[:, m * 512:(m + 1) * 512],
                                     wt[:, ko, :, m * P:(m + 1) * P],
                                     x8_tiles[qtr][:, ko, :, :],
                                     start=(ko == 0), stop=(ko == KO4 - 1),
                                     perf_mode=DR)
            # head-pair 0 (psum parts 0:64) on DVE in one op; head-pair 1
            # (parts 64:128, partition-shifted) on ACT as two per-m ops
            nc.vector.tensor_tensor(
                dst[0][:, :, sq],
                ppt[0:64, :].rearrange("p (m n) -> p m n", n=512),
                bt[0:64, :].rearrange("p (m j) -> p m j", j=1)
                .broadcast_to([64, MT, 512]),
                mybir.AluOpType.add)
            for m in range(MT):
                nc.scalar.activation(
                    dst[1][:, m, sq], ppt[64:P, m * 512:(m + 1) * 512],
                    mybir.ActivationFunctionType.Identity,
                    bias=bt[64:P, m:m + 1], scale=1.0)

        def v_proj(qtr, st2):
            if st2 == 0:
                v_proj.pvt = ps.tile([P, 1024], F32, tag="sp", bufs=2,
                                     name="pvt")
            pvt = v_proj.pvt
            for st in (2 * st2, 2 * st2 + 1):
                pv = pvt[:, st * W:(st + 1) * W]
                for ko in range(KO8):
                    nc.tensor.matmul(pv, xq_tiles[qtr][:, ko, st * P:(st + 1) * P],
                                     wv_t[:, ko, :], start=(ko == 0),
                                     stop=(ko == KO8 - 1))

        def v_finish(qtr):
            pvt = v_proj.pvt
            vb = cp.tile([P, 1024], BF, tag="vb", bufs=2, name="vb")
            nc.scalar.activation(vb[:], pvt[:],
                                 mybir.ActivationFunctionType.Copy,
                                 bias=0.0, scale=1.0)
            vbv = vb[:].rearrange("p (a b h c) -> p a b h c", a=2, b=2, c=64)
            Av = vaugA_t[:, 2 * qtr:2 * qtr + 2, :, :].rearrange(
                "p a b (h c) -> p a b h c", c=HW)[:, :, :, :, 0:64]
            Rv = vaugR_t[:, 2 * qtr:2 * qtr + 2, :, :].rearrange(
                "p a b (h c) -> p a b h c", c=HW)[:, :, :, :, 0:64]
            nc.gpsimd.tensor_tensor(Av, vbv,
                                    zt[:].rearrange("p (a b h c) -> p a b h c",
                                                    a=1, b=1, h=1)
                                    .broadcast_to([P, 2, 2, 4, 64]),
                                    mybir.AluOpType.add)
            nc.gpsimd.tensor_tensor(Rv, vbv, Av, mybir.AluOpType.subtract)

        class Block:
            def __init__(self, q, h):
                _uid[0] += 1
                self.u = _uid[0]
                self.q, self.h = q, h
                self.sq = slice(q * 512, (q + 1) * 512)
                self.hq = slice(32 * (h % 2), 32 * (h % 2) + 32)
                self.qt8, self.kt8 = qt8_m[h // 2], kt8_m[h // 2]
                self.hc = slice(h * HW, (h + 1) * HW)
                self.pairs = 0          # score/exp steps emitted
                self.pts = [None] * NPAIR
                self.pv_done = 0
                self.oPf = None
                self.done = False

            def can_score(self, qtr):
                return (self.pairs < NPAIR and self.q <= qtr
                        and self.pairs // 2 <= qtr)

            def step_score(self):
                i = self.pairs
                pt = ptp.tile([P, 2, 512], F8, tag="pt", bufs=26,
                              name=f"pt{self.u}_{i}")
                sp = ps.tile([P, 1024], F32, tag="sp", bufs=2,
                             name=f"sp{self.u}_{i}")
                for half in (0, 1):
                    ks = slice((2 * i + half) * P, (2 * i + half + 1) * P)
                    nc.tensor.matmul(sp[:, half * 512:(half + 1) * 512],
                                     self.kt8[self.hq, :, ks],
                                     self.qt8[self.hq, :, self.sq],
                                     start=True, stop=True, perf_mode=DR)
                ptf = pt[:].rearrange("p t n -> p (t n)")
                if ea.next() == "A":
                    nc.scalar.activation(ptf, sp[:],
                                         mybir.ActivationFunctionType.Exp,
                                         bias=0.0, scale=SCALE)
                else:
                    nc.vector.tensor_scalar(ptf.bitcast(I8), sp[:],
                                            A_SCH, B_SCH,
                                            mybir.AluOpType.mult,
                                            mybir.AluOpType.add)
                self.pts[i] = pt
                self.pairs += 1

            def grant(self):
                self.oPf = ps.tile([P, 512], F32, tag="oPf", bufs=3,
                                   name=f"oPf{self.u}")

            def pv_ready(self, vfin_qtr):
                # PV pair i needs exp lag 2 and vaug through qtr i//2
                i = self.pv_done
                return (self.oPf is not None and i < NPAIR
                        and (i <= self.pairs - 3 or self.pairs == NPAIR)
                        and i // 2 <= vfin_qtr)

            def step_pv(self):
                i = self.pv_done
                oP = self.oPf[:].rearrange("p (c n) -> p c n", n=128)[:, :, 0:HW]
                pt = self.pts[i]
                for c in range(4):
                    stat = pt[:, :, c * 128:(c + 1) * 128]
                    nc.tensor.matmul(oP[:, c, :], stat,
                                     vaugA_t[:, i, :, self.hc],
                                     start=(i == 0), stop=False,
                                     perf_mode=DR)
                    nc.tensor.matmul(oP[:, c, :], stat,
                                     vaugR_t[:, i, :, self.hc],
                                     start=False, stop=(i == NPAIR - 1),
                                     perf_mode=DR)
                self.pts[i] = None
                self.pv_done = i + 1

            def epilogue(self):
                oP = self.oPf[:].rearrange("p (c n) -> p c n", n=128)[:, :, 0:HW]
                attnP = get_attnP(self.q)
                h = self.h
                rs = cp.tile([P, 4], F32, tag="rs", bufs=4, name=f"rs{self.u}")
                nc.vector.reciprocal(
                    rs[:], oP[:, :, 64:HW].rearrange("p c j -> p (c j)"))
                nc.vector.tensor_tensor(
                    attnP[:, :, h // 2, 64 * (h % 2):64 * (h % 2) + 64],
                    oP[:, :, 0:64],
                    rs[:].rearrange("p (c j) -> p c j", j=1)
                    .broadcast_to([P, 4, 64]),
                    mybir.AluOpType.mult)
                self.done = True
                self.oPf = None
                if h % 2 == 1:
                    pending_tp.extend(
                        (self.q, h // 2, c, attnP) for c in range(4))

        # ---------------- unified schedule ----------------
        proj_items = []
        for t in range(NQ):
            proj_items.append([
                ("ldx", t), ("qkq", t), ("qkk", t),
                ("v0", t), ("v1", t), ("vfin", t)])

        def emit_proj(kind, t):
            if kind == "ldx":
                if t > 1:
                    load_xq(t)
            elif kind == "qkq":
                qk_proj(t, w8q_t, bq_t, qt8_m)
            elif kind == "qkk":
                qk_proj(t, w8k_t, bk_t, kt8_m)
            elif kind == "v0":
                v_proj(t, 0)
            elif kind == "v1":
                v_proj(t, 1)
            elif kind == "vfin":
                v_finish(t)
                vfin_done[0] = t

        vfin_done = [-1]
        all_blocks = [Block(q, h) for q in range(NQ) for h in range(4)]
        unstarted = list(all_blocks)
        active = []
        MAXA = 5
        SLOTS = 3
        slots_used = [0]
        qtr = [-1]
        pq = []   # flat pending proj emissions

        def advance_qtr():
            if qtr[0] < NQ - 1:
                qtr[0] += 1
                pq.extend(proj_items[qtr[0]])
                return True
            return False

        advance_qtr()
        while pq:
            k, t = pq.pop(0)
            emit_proj(k, t)

        def refill():
            while (len(active) < MAXA and unstarted
                   and unstarted[0].q <= qtr[0]):
                active.append(unstarted.pop(0))

        def pump_pv():
            for b in active:
                while b.oPf is not None and b.pv_ready(vfin_done[0]):
                    b.step_pv()
                if (b.oPf is not None and b.pv_done == NPAIR
                        and not b.done):
                    b.epilogue()
                    slots_used[0] -= 1
            while [b for b in active if b.done]:
                active.remove([b for b in active if b.done][0])

        def grant_slots():
            for b in active:
                if slots_used[0] >= SLOTS:
                    break
                if b.oPf is None and not b.done and b.pairs > 0:
                    b.grant()
                    slots_used[0] += 1

        tick = [0]
        while True:
            refill()
            grant_slots()
            cand = next((b for b in active if b.can_score(qtr[0])), None)
            if cand is None:
                if pq:
                    k, t = pq.pop(0)
                    emit_proj(k, t)
                    continue
                if advance_qtr():
                    continue
                if not active and not unstarted:
                    break
                # blocks waiting on pv/epilogue only
                pump_pv()
                grant_slots()
                if pending_tp:
                    emit_tp(*pending_tp.pop(0))
                elif pending_out:
                    outproj_unit(*pending_out.pop(0))
                elif pending_dma:
                    emit_dma(*pending_dma.pop(0))
                continue
            cand.step_score()
            tick[0] += 1
            pump_pv()
            if pending_tp:
                emit_tp(*pending_tp.pop(0))
            elif pending_dma:
                emit_dma(*pending_dma.pop(0))
            if pending_out and tick[0] % 2 == 0:
                outproj_unit(*pending_out.pop(0))
        while pending_tp:
            emit_tp(*pending_tp.pop(0))
        while pending_out:
            q, dc, ceng = pending_out.pop(0)
            outproj_unit(q, dc, "D" if len(pending_out) == 0 else ceng)
        while pending_dma:
            emit_dma(*pending_dma.pop(0))
    nc.compile()
    return nc


def _prep_inputs(x, Wq, bq, Wk, bk, Wv, bv, Wo, bo):
    # straight perm: psum partition p of m-tile m holds W-col
    # (p//32)*64 + 32*m + (p%32)  (head p//32, dk-dim 32m + p%32)
    perm = np.empty(W, dtype=np.int64)
    for m in range(MT):
        p = np.arange(P)
        perm[m * P + p] = (p // 32) * 64 + 32 * m + (p % 32)

    in_maps = []
    for c in range(8):
        b, g = c // 4, c % 4
        cs = slice(g * W, (g + 1) * W)
        xTb = np.ascontiguousarray(x[b].T)
        Wq_l, bq_l = Wq[:, cs][:, perm], bq[cs][perm]
        Wk_l, bk_l = Wk[:, cs][:, perm], bk[cs][perm]
        in_maps.append({
            "xb": xTb.astype(BF16),
            "x8": xTb.astype(E4),
            "wq8": (16.0 * Wq_l).astype(E4),
            "wk8": (16.0 * Wk_l).astype(E4),
            "wv": (16.0 * Wv[:, cs]).astype(BF16),
            "wo": Wo[cs, :].astype(BF16),
            "bq": np.ascontiguousarray(16.0 * bq_l.reshape(MT, P).T),
            "bk": np.ascontiguousarray(16.0 * bk_l.reshape(MT, P).T),
        })
    return in_maps


def kernel(x, Wq, bq, Wk, bk, Wv, bv, Wo, bo):
    x = np.asarray(x, dtype=np.float32)
    Wq, bq = np.asarray(Wq, np.float32), np.asarray(bq, np.float32)
    Wk, bk = np.asarray(Wk, np.float32), np.asarray(bk, np.float32)
    Wv, bv = np.asarray(Wv, np.float32), np.asarray(bv, np.float32)
    Wo, bo = np.asarray(Wo, np.float32), np.asarray(bo, np.float32)

    if "nc" not in _CACHE:
        _CACHE["nc"] = build_nc()
    nc = _CACHE["nc"]

    in_maps = _prep_inputs(x, Wq, bq, Wk, bk, Wv, bv, Wo, bo)
    res = run_bass_kernel_spmd(nc, in_maps, core_ids=list(range(8))).results

    extra = bv @ Wo + bo   # bv folded out of the V projection
    out = np.empty((B, S, D), dtype=np.float32)
    for b in range(B):
        acc = res[4 * b]["out"].astype(np.float32)
        for g in range(1, 4):
            acc += res[4 * b + g]["out"].astype(np.float32)
        out[b] = acc.T + extra
    return out


# revision 14
# speedup vs baseline: 1.1739x; 1.1739x over previous
"""Multi-head attention (B=2, S=2048, D=1024, H=16, Dk=64) on 8 NeuronCores.

Sharding: 2-way data parallel over batch x 4-way tensor parallel over heads.
Core c = 4*b + g handles batch b, head group g (4 heads = 256 cols).
W_o is row-sliced; host sums the 4 partial outputs per batch (+ bv@Wo + bo).

v3 (elementwise-bound rework; ACT/DVE are the binding engines):
  - Q/K projections: fp8 DoubleRow as before, but the W-column permutation is
    chosen so each m-tile's psum [128,512] maps partition-straight into a
    single [128,2,S] qt8/kt8 tile (head h at partition base 32h, legal
    quadrant bases). The psum->sbuf move is ONE full-partition op per
    projection: DVE tensor_tensor add with a broadcast per-(p,m) bias tile;
    no /16 rescale (scales folded into the exp constant).
  - scores: per-head DR with dk=64 at quadrant base 32h (unchanged math).
  - softmax exp: ACT exact Exp and DVE Schraudolph split by weighted
    round-robin, scale = 0.125/256.
  - P@V FLIPPED: stationary = probs chunk [128,2,128], moving = vaug
    [128,2,65] (64 V-dims + 1 rowsum const col per head); out psum
    [128 tok, 4, 65]. Full A+R residual compensation (V stored as fp8 A
    plus equal-scale fp8 residual R). 65-wide outputs cut P@V PE time by
    ~35% and shrink the normalization to [128,4,64] broadcast ops.
  - normalization: DVE reciprocal [128,4] + one broadcast tensor_tensor mul
    writing bf16 attnP; DMA-transpose ([128,128] bf16 tiles, 14ns/xbar-tile)
    rebuilds the [dims, tok] layout for the output projection.
  - V path: V-proj psum -> ONE bf16 copy per qtr; Pool (gpsimd) builds fp8
    A and R from SBUF via tensor_tensor (GPSIMD cannot touch PSUM).
  - output projection: flipped (out^T [D,S] in DRAM, host transposes),
    moving = ot_t [128,2,512] bf16, stationary = Wo; psum->sbuf copies on
    ACT. bq/bk biases on-chip; bv/bo folded into the host-side reduction.
"""
import numpy as np
import ml_dtypes
from contextlib import ExitStack

import concourse.bass as bass
import concourse.mybir as mybir
import concourse.tile as tile
from concourse import bacc
from concourse.bass_utils import run_bass_kernel_spmd

F32 = mybir.dt.float32
F8 = mybir.dt.float8e4
I8 = mybir.dt.int8
BF = mybir.dt.bfloat16
E4 = ml_dtypes.float8_e4m3
BF16 = ml_dtypes.bfloat16
DR = mybir.MatmulPerfMode.DoubleRow

B, S, D = 2, 2048, 1024
P = 128
W = 256              # local width (4 heads x 64)
MT = 2               # m-tiles of 128
KO8 = D // P         # 8 bf16 contraction ktiles
KO4 = D // 256       # 4 DoubleRow ktiles
NQ = S // 512        # 4 sq chunks
NPAIR = S // 256     # 8 sk chunk-pairs
HW = 65              # per-head moving width in P@V (64 dims + rowsum col)
SCALE = 0.125 / 256.0          # qt8/kt8 carry x16 each
A_SCH = 8.0 / np.log(2.0) * SCALE
B_SCH = 55.6

_CACHE = {}


# exp engine split: deterministic weighted round-robin (Bresenham).
class _ExpAssign:
    def __init__(self, wa, wd):
        self.w = {"A": wa, "D": wd}
        self.acc = {"A": 0.0, "D": 0.0}

    def next(self):
        for k in self.acc:
            self.acc[k] += self.w[k]
        k = max(self.acc, key=lambda e: self.acc[e])
        self.acc[k] -= sum(self.w.values())
        return k


def build_nc():
    nc = bacc.Bacc("TRN2", target_bir_lowering=False, debug=False, num_devices=8)
    xb = nc.dram_tensor("xb", [D, S], BF, kind="ExternalInput").ap()
    x8 = nc.dram_tensor("x8", [D, S], F8, kind="ExternalInput").ap()
    wq8 = nc.dram_tensor("wq8", [D, W], F8, kind="ExternalInput").ap()
    wk8 = nc.dram_tensor("wk8", [D, W], F8, kind="ExternalInput").ap()
    wv = nc.dram_tensor("wv", [D, W], BF, kind="ExternalInput").ap()
    wo = nc.dram_tensor("wo", [W, D], BF, kind="ExternalInput").ap()
    bq = nc.dram_tensor("bq", [P, MT], F32, kind="ExternalInput").ap()
    bk = nc.dram_tensor("bk", [P, MT], F32, kind="ExternalInput").ap()
    out = nc.dram_tensor("out", [D, S], BF, kind="ExternalOutput").ap()

    x8_r = x8.rearrange("(ko t p) s -> p ko t s", p=P, t=2)
    xb_r = xb.rearrange("(ko p) s -> p ko s", p=P)
    wq8_r = wq8.rearrange("(ko t p) w -> p ko t w", p=P, t=2)
    wk8_r = wk8.rearrange("(ko t p) w -> p ko t w", p=P, t=2)
    wv_r = wv.rearrange("(ko p) w -> p ko w", p=P)
    wo_r = wo.rearrange("(m p) d -> p m d", p=P)

    with tile.TileContext(nc) as tc, ExitStack() as ctx:
        sb = ctx.enter_context(tc.tile_pool(name="sb", bufs=1))
        xp = ctx.enter_context(tc.tile_pool(name="xp", bufs=2))
        ptp = ctx.enter_context(tc.tile_pool(name="ptp", bufs=4))
        cp = ctx.enter_context(tc.tile_pool(name="cp", bufs=8))
        ps = ctx.enter_context(tc.tile_pool(name="ps", bufs=1, space="PSUM"))

        # ---- resident inputs (issue order = DMA priority) ----
        w8q_t = sb.tile([P, KO4, 2, W], F8)
        nc.scalar.dma_start(w8q_t[:], wq8_r)
        x8_tiles = [xp.tile([P, KO4, 2, 512], F8, tag="x8", name=f"x8_{i}",
                            bufs=4) for i in range(4)]

        def load_x8(qtr):
            sq = slice(qtr * 512, (qtr + 1) * 512)
            nc.sync.dma_start(x8_tiles[qtr][:, 0:2, :, :], x8_r[:, 0:2, :, sq])
            nc.sync.dma_start(x8_tiles[qtr][:, 2:4, :, :], x8_r[:, 2:4, :, sq])

        sq0 = slice(0, 512)
        nc.sync.dma_start(x8_tiles[0][:, 0:2, :, :], x8_r[:, 0:2, :, sq0])
        bq_t = sb.tile([P, MT], F32)
        nc.sync.dma_start(bq_t[:], bq)
        nc.sync.dma_start(x8_tiles[0][:, 2:4, :, :], x8_r[:, 2:4, :, sq0])
        w8k_t = sb.tile([P, KO4, 2, W], F8)
        nc.scalar.dma_start(w8k_t[:], wk8_r)
        bk_t = sb.tile([P, MT], F32)
        nc.sync.dma_start(bk_t[:], bk)

        xq_tiles = [None] * 4

        def load_xq(qtr):
            sq = slice(qtr * 512, (qtr + 1) * 512)
            xq_tiles[qtr] = xp.tile([P, KO8, 512], BF, tag="xq", bufs=2,
                                    name=f"xq{qtr}")
            nc.sync.dma_start(xq_tiles[qtr][:, 0:KO8 // 2, :],
                              xb_r[:, 0:KO8 // 2, sq])
            nc.sync.dma_start(xq_tiles[qtr][:, KO8 // 2:KO8, :],
                              xb_r[:, KO8 // 2:KO8, sq])

        load_xq(0)
        wv_t = sb.tile([P, KO8, W], BF)
        nc.sync.dma_start(wv_t[:], wv_r)
        for _q in (1, 2, 3):
            load_x8(_q)
        load_xq(1)
        wo_t = sb.tile([P, MT, D], BF)
        nc.sync.dma_start(wo_t[:], wo_r)

        zt = sb.tile([P, 1], BF)
        nc.vector.memset(zt[:], 0.0)

        # qt8/kt8: two [64, m, S] tiles (head-pairs); within a tile head
        # h%2 sits at base 32*(h%2) (base 96 is illegal for matmul operands),
        # dk = 32m + p%32
        qt8_m = [sb.tile([64, MT, S], F8, name=f"qt8_{j}") for j in range(2)]
        kt8_m = [sb.tile([64, MT, S], F8, name=f"kt8_{j}") for j in range(2)]
        # vaug A/R: [sk-part, pair, parity, head*65]; per head 64 V-dims
        # then 1 rowsum col (A: 16.0; R: 0.0)
        vaugA_t = sb.tile([P, NPAIR, 2, 4 * HW], F8)
        vaugR_t = sb.tile([P, NPAIR, 2, 4 * HW], F8)
        consA = vaugA_t[:].rearrange("p i t (h c) -> p i t h c", c=HW)[:, :, :, :, 64:HW]
        nc.gpsimd.memset(consA, 16.0)
        consR = vaugR_t[:].rearrange("p i t (h c) -> p i t h c", c=HW)[:, :, :, :, 64:HW]
        nc.gpsimd.memset(consR, 0.0)
        ot_t = sb.tile([P, MT, S], BF)

        ea_p1 = _ExpAssign(1.0, 1.0)      # phase-1 blocks
        ea_p2 = _ExpAssign(1.05, 1.0)     # phase-2 mix

        _uid = [0]
        pending_out = []
        pending_tp = []
        pending_dma = []

        ea = _ExpAssign(1.08, 1.0)

        attnP_tiles = {}

        def get_attnP(q):
            if q not in attnP_tiles:
                attnP_tiles[q] = cp.tile([P, 4, 2, P], BF, tag="attnP",
                                         bufs=2, name=f"attnP{q}")
            return attnP_tiles[q]

        def emit_tp(q, hp, c, attnP):
            nc.sync.dma_start(
                ot_t[:, hp, q * 512 + c * 128: q * 512 + (c + 1) * 128],
                attnP[:, c, hp, :], transpose=True)
            if hp == 1 and c == 3:
                outproj(q)

        def emit_dma(q, dc0, ob):
            sq = slice(q * 512, (q + 1) * 512)
            dr = out[dc0 * P:(dc0 + 2) * P, sq].rearrange(
                "(a p) n -> p a n", a=2)
            nc.sync.dma_start(dr, ob[:])

        _ob_cur = [None]

        def outproj_unit(q, dc, ceng):
            pot = ps.tile([P, 1024], F32, tag="sp", bufs=3, name="pot")
            po = pot[:, 0:512]
            sq = slice(q * 512, (q + 1) * 512)
            for m in range(MT):
                nc.tensor.matmul(po[:], wo_t[:, m, dc * P:(dc + 1) * P],
                                 ot_t[:, m, sq],
                                 start=(m == 0), stop=(m == MT - 1))
            if dc % 2 == 0:
                _ob_cur[0] = cp.tile([P, 2, 512], BF, tag="ob", bufs=3, name="ob")
            ob = _ob_cur[0]
            half = ob[:, dc % 2, :]
            if ceng == "A":
                nc.scalar.activation(half, po[:],
                                     mybir.ActivationFunctionType.Copy,
                                     bias=0.0, scale=1.0)
            else:
                nc.vector.tensor_copy(half, po[:])
            if dc % 2 == 1:
                pending_dma.append((q, dc - 1, ob))

        def outproj(q):
            for dc in range(8):
                pending_out.append((q, dc, "A" if dc % 4 != 3 else "D"))

        def qk_proj(qtr, wt, bt, dst):
            sq = slice(qtr * 512, (qtr + 1) * 512)
            ppt = ps.tile([P, 1024], F32, tag="sp", bufs=3, name="ppt")
            for m in range(MT):
                for ko in range(KO4):
                    nc.tensor.matmul(ppt[:, m * 512:(m + 1) * 512],
                                     wt[:, ko, :, m * P:(m + 1) * P],
                                     x8_tiles[qtr][:, ko, :, :],
                                     start=(ko == 0), stop=(ko == KO4 - 1),
                                     perf_mode=DR)
            # head-pair 0 (psum parts 0:64) on DVE in one op; head-pair 1
            # (parts 64:128, partition-shifted) on ACT as two per-m ops
            nc.vector.tensor_tensor(
                dst[0][:, :, sq],
                ppt[0:64, :].rearrange("p (m n) -> p m n", n=512),
                bt[0:64, :].rearrange("p (m j) -> p m j", j=1)
                .broadcast_to([64, MT, 512]),
                mybir.AluOpType.add)
            for m in range(MT):
                nc.scalar.activation(
                    dst[1][:, m, sq], ppt[64:P, m * 512:(m + 1) * 512],
                    mybir.ActivationFunctionType.Identity,
                    bias=bt[64:P, m:m + 1], scale=1.0)

        def v_proj(qtr, st2):
            if st2 == 0:
                v_proj.pvt = ps.tile([P, 1024], F32, tag="sp", bufs=3,
                                     name="pvt")
            pvt = v_proj.pvt
            for st in (2 * st2, 2 * st2 + 1):
                pv = pvt[:, st * W:(st + 1) * W]
                for ko in range(KO8):
                    nc.tensor.matmul(pv, xq_tiles[qtr][:, ko, st * P:(st + 1) * P],
                                     wv_t[:, ko, :], start=(ko == 0),
                                     stop=(ko == KO8 - 1))

        def v_finish(qtr):
            pvt = v_proj.pvt
            vb = cp.tile([P, 1024], BF, tag="vb", bufs=2, name="vb")
            nc.scalar.activation(vb[:], pvt[:],
                                 mybir.ActivationFunctionType.Copy,
                                 bias=0.0, scale=1.0)
            vbv = vb[:].rearrange("p (a b h c) -> p a b h c", a=2, b=2, c=64)
            Av = vaugA_t[:, 2 * qtr:2 * qtr + 2, :, :].rearrange(
                "p a b (h c) -> p a b h c", c=HW)[:, :, :, :, 0:64]
            Rv = vaugR_t[:, 2 * qtr:2 * qtr + 2, :, :].rearrange(
                "p a b (h c) -> p a b h c", c=HW)[:, :, :, :, 0:64]
            nc.gpsimd.tensor_tensor(Av, vbv,
                                    zt[:].rearrange("p (a b h c) -> p a b h c",
                                                    a=1, b=1, h=1)
                                    .broadcast_to([P, 2, 2, 4, 64]),
                                    mybir.AluOpType.add)
            nc.gpsimd.tensor_tensor(Rv, vbv, Av, mybir.AluOpType.subtract)

        class Block:
            def __init__(self, q, h):
                _uid[0] += 1
                self.u = _uid[0]
                self.q, self.h = q, h
                self.sq = slice(q * 512, (q + 1) * 512)
                self.hq = slice(32 * (h % 2), 32 * (h % 2) + 32)
                self.qt8, self.kt8 = qt8_m[h // 2], kt8_m[h // 2]
                self.hc = slice(h * HW, (h + 1) * HW)
                self.pairs = 0          # score/exp steps emitted
                self.pts = [None] * NPAIR
                self.pv_done = 0
                self.oPf = None
                self.done = False

            def can_score(self, qtr):
                return (self.pairs < NPAIR and self.q <= qtr
                        and self.pairs // 2 <= qtr)

            def step_score(self):
                i = self.pairs
                pt = ptp.tile([P, 2, 512], F8, tag="pt", bufs=26,
                              name=f"pt{self.u}_{i}")
                sp = ps.tile([P, 1024], F32, tag="sp", bufs=3,
                             name=f"sp{self.u}_{i}")
                for half in (0, 1):
                    ks = slice((2 * i + half) * P, (2 * i + half + 1) * P)
                    nc.tensor.matmul(sp[:, half * 512:(half + 1) * 512],
                                     self.kt8[self.hq, :, ks],
                                     self.qt8[self.hq, :, self.sq],
                                     start=True, stop=True, perf_mode=DR)
                ptf = pt[:].rearrange("p t n -> p (t n)")
                if ea.next() == "A":
                    nc.scalar.activation(ptf, sp[:],
                                         mybir.ActivationFunctionType.Exp,
                                         bias=0.0, scale=SCALE)
                else:
                    nc.vector.tensor_scalar(ptf.bitcast(I8), sp[:],
                                            A_SCH, B_SCH,
                                            mybir.AluOpType.mult,
                                            mybir.AluOpType.add)
                self.pts[i] = pt
                self.pairs += 1

            def grant(self):
                self.oPf = ps.tile([P, 512], F32, tag="oPf", bufs=2,
                                   name=f"oPf{self.u}")

            def pv_ready(self, vfin_qtr):
                # PV pair i needs exp lag 2 and vaug through qtr i//2
                i = self.pv_done
                return (self.oPf is not None and i < NPAIR
                        and (i <= self.pairs - 3 or self.pairs == NPAIR)
                        and i // 2 <= vfin_qtr)

            def step_pv(self):
                i = self.pv_done
                oP = self.oPf[:].rearrange("p (c n) -> p c n", n=128)[:, :, 0:HW]
                pt = self.pts[i]
                for c in range(4):
                    stat = pt[:, :, c * 128:(c + 1) * 128]
                    nc.tensor.matmul(oP[:, c, :], stat,
                                     vaugA_t[:, i, :, self.hc],
                                     start=(i == 0), stop=False,
                                     perf_mode=DR)
                    nc.tensor.matmul(oP[:, c, :], stat,
                                     vaugR_t[:, i, :, self.hc],
                                     start=False, stop=(i == NPAIR - 1),
                                     perf_mode=DR)
                self.pts[i] = None
                self.pv_done = i + 1

            def epilogue(self):
                oP = self.oPf[:].rearrange("p (c n) -> p c n", n=128)[:, :, 0:HW]
                attnP = get_attnP(self.q)
                h = self.h
                rs = cp.tile([P, 4], F32, tag="rs", bufs=4, name=f"rs{self.u}")
                nc.vector.reciprocal(
                    rs[:], oP[:, :, 64:HW].rearrange("p c j -> p (c j)"))
                nc.vector.tensor_tensor(
                    attnP[:, :, h // 2, 64 * (h % 2):64 * (h % 2) + 64],
                    oP[:, :, 0:64],
                    rs[:].rearrange("p (c j) -> p c j", j=1)
                    .broadcast_to([P, 4, 64]),
                    mybir.AluOpType.mult)
                self.done = True
                self.oPf = None
                if h % 2 == 1:
                    pending_tp.extend(
                        (self.q, h // 2, c, attnP) for c in range(4))

        # ---------------- unified schedule ----------------
        proj_items = []
        for t in range(NQ):
            proj_items.append([
                ("ldx", t), ("qkq", t), ("qkk", t),
                ("v0", t), ("v1", t), ("vfin", t)])

        def emit_proj(kind, t):
            if kind == "ldx":
                if t > 1:
                    load_xq(t)
            elif kind == "qkq":
                qk_proj(t, w8q_t, bq_t, qt8_m)
            elif kind == "qkk":
                qk_proj(t, w8k_t, bk_t, kt8_m)
            elif kind == "v0":
                v_proj(t, 0)
            elif kind == "v1":
                v_proj(t, 1)
            elif kind == "vfin":
                v_finish(t)
                vfin_done[0] = t

        vfin_done = [-1]
        all_blocks = [Block(q, h) for q in range(NQ) for h in range(4)]
        unstarted = list(all_blocks)
        active = []
        MAXA = 5
        SLOTS = 2
        slots_used = [0]
        qtr = [-1]
        pq = []   # flat pending proj emissions

        def advance_qtr():
            if qtr[0] < NQ - 1:
                qtr[0] += 1
                pq.extend(proj_items[qtr[0]])
                return True
            return False

        advance_qtr()
        while pq:
            k, t = pq.pop(0)
            emit_proj(k, t)

        def refill():
            while (len(active) < MAXA and unstarted
                   and unstarted[0].q <= qtr[0]):
                active.append(unstarted.pop(0))

        def pump_pv(cap=1):
            n = 0
            for b in active:
                while (b.oPf is not None and b.pv_ready(vfin_done[0])
                       and (cap is None or n < cap)):
                    b.step_pv()
                    n += 1
                if (b.oPf is not None and b.pv_done == NPAIR
                        and not b.done):
                    b.epilogue()
                    slots_used[0] -= 1
            while [b for b in active if b.done]:
                active.remove([b for b in active if b.done][0])

        def grant_slots():
            for b in active:
                if slots_used[0] >= SLOTS:
                    break
                if b.oPf is None and not b.done and b.pairs > 0:
                    b.grant()
                    slots_used[0] += 1

        tick = [0]
        while True:
            refill()
            grant_slots()
            cand = next((b for b in active if b.can_score(qtr[0])), None)
            if cand is None:
                if pq:
                    k, t = pq.pop(0)
                    emit_proj(k, t)
                    continue
                if advance_qtr():
                    continue
                if not active and not unstarted:
                    break
                # blocks waiting on pv/epilogue only
                pump_pv(cap=None)
                grant_slots()
                if pending_tp:
                    emit_tp(*pending_tp.pop(0))
                elif pending_out:
                    outproj_unit(*pending_out.pop(0))
                elif pending_dma:
                    emit_dma(*pending_dma.pop(0))
                continue
            cand.step_score()
            tick[0] += 1
            pump_pv()
            if pending_tp:
                emit_tp(*pending_tp.pop(0))
            elif pending_dma:
                emit_dma(*pending_dma.pop(0))
            if pending_out and tick[0] % 2 == 0:
                outproj_unit(*pending_out.pop(0))
        while pending_tp:
            emit_tp(*pending_tp.pop(0))
        while pending_out:
            q, dc, ceng = pending_out.pop(0)
            outproj_unit(q, dc, "D" if len(pending_out) == 0 else ceng)
        while pending_dma:
            emit_dma(*pending_dma.pop(0))
    nc.compile()
    return nc


def _prep_inputs(x, Wq, bq, Wk, bk, Wv, bv, Wo, bo):
    # straight perm: psum partition p of m-tile m holds W-col
    # (p//32)*64 + 32*m + (p%32)  (head p//32, dk-dim 32m + p%32)
    perm = np.empty(W, dtype=np.int64)
    for m in range(MT):
        p = np.arange(P)
        perm[m * P + p] = (p // 32) * 64 + 32 * m + (p % 32)

    in_maps = []
    for c in range(8):
        b, g = c // 4, c % 4
        cs = slice(g * W, (g + 1) * W)
        xTb = np.ascontiguousarray(x[b].T)
        Wq_l, bq_l = Wq[:, cs][:, perm], bq[cs][perm]
        Wk_l, bk_l = Wk[:, cs][:, perm], bk[cs][perm]
        in_maps.append({
            "xb": xTb.astype(BF16),
            "x8": xTb.astype(E4),
            "wq8": (16.0 * Wq_l).astype(E4),
            "wk8": (16.0 * Wk_l).astype(E4),
            "wv": (16.0 * Wv[:, cs]).astype(BF16),
            "wo": Wo[cs, :].astype(BF16),
            "bq": np.ascontiguousarray(16.0 * bq_l.reshape(MT, P).T),
            "bk": np.ascontiguousarray(16.0 * bk_l.reshape(MT, P).T),
        })
    return in_maps


def kernel(x, Wq, bq, Wk, bk, Wv, bv, Wo, bo):
    x = np.asarray(x, dtype=np.float32)
    Wq, bq = np.asarray(Wq, np.float32), np.asarray(bq, np.float32)
    Wk, bk = np.asarray(Wk, np.float32), np.asarray(bk, np.float32)
    Wv, bv = np.asarray(Wv, np.float32), np.asarray(bv, np.float32)
    Wo, bo = np.asarray(Wo, np.float32), np.asarray(bo, np.float32)

    if "nc" not in _CACHE:
        _CACHE["nc"] = build_nc()
    nc = _CACHE["nc"]

    in_maps = _prep_inputs(x, Wq, bq, Wk, bk, Wv, bv, Wo, bo)
    res = run_bass_kernel_spmd(nc, in_maps, core_ids=list(range(8))).results

    extra = bv @ Wo + bo   # bv folded out of the V projection
    out = np.empty((B, S, D), dtype=np.float32)
    for b in range(B):
        acc = res[4 * b]["out"].astype(np.float32)
        for g in range(1, 4):
            acc += res[4 * b + g]["out"].astype(np.float32)
        out[b] = acc.T + extra
    return out


# revision 15
# speedup vs baseline: 1.2053x; 1.0268x over previous
"""Multi-head attention (B=2, S=2048, D=1024, H=16, Dk=64) on 8 NeuronCores.

Sharding: 2-way data parallel over batch x 4-way tensor parallel over heads.
Core c = 4*b + g handles batch b, head group g (4 heads = 256 cols).
W_o is row-sliced; host sums the 4 partial outputs per batch (+ bv@Wo + bo).

v3 (elementwise-bound rework; ACT/DVE are the binding engines):
  - Q/K projections: fp8 DoubleRow as before, but the W-column permutation is
    chosen so each m-tile's psum [128,512] maps partition-straight into a
    single [128,2,S] qt8/kt8 tile (head h at partition base 32h, legal
    quadrant bases). The psum->sbuf move is ONE full-partition op per
    projection: DVE tensor_tensor add with a broadcast per-(p,m) bias tile;
    no /16 rescale (scales folded into the exp constant).
  - scores: per-head DR with dk=64 at quadrant base 32h (unchanged math).
  - softmax exp: ACT exact Exp and DVE Schraudolph split by weighted
    round-robin, scale = 0.125/256.
  - P@V FLIPPED: stationary = probs chunk [128,2,128], moving = vaug
    [128,2,65] (64 V-dims + 1 rowsum const col per head); out psum
    [128 tok, 4, 65]. Full A+R residual compensation (V stored as fp8 A
    plus equal-scale fp8 residual R). 65-wide outputs cut P@V PE time by
    ~35% and shrink the normalization to [128,4,64] broadcast ops.
  - normalization: DVE reciprocal [128,4] + one broadcast tensor_tensor mul
    writing bf16 attnP; DMA-transpose ([128,128] bf16 tiles, 14ns/xbar-tile)
    rebuilds the [dims, tok] layout for the output projection.
  - V path: V-proj psum -> ONE bf16 copy per qtr; Pool (gpsimd) builds fp8
    A and R from SBUF via tensor_tensor (GPSIMD cannot touch PSUM).
  - output projection: flipped (out^T [D,S] in DRAM, host transposes),
    moving = ot_t [128,2,512] bf16, stationary = Wo; psum->sbuf copies on
    ACT. bq/bk biases on-chip; bv/bo folded into the host-side reduction.
"""
import numpy as np
import ml_dtypes
from contextlib import ExitStack

import concourse.bass as bass
import concourse.mybir as mybir
import concourse.tile as tile
from concourse import bacc
from concourse.bass_utils import run_bass_kernel_spmd

F32 = mybir.dt.float32
F8 = mybir.dt.float8e4
I8 = mybir.dt.int8
BF = mybir.dt.bfloat16
E4 = ml_dtypes.float8_e4m3
BF16 = ml_dtypes.bfloat16
DR = mybir.MatmulPerfMode.DoubleRow

B, S, D = 2, 2048, 1024
P = 128
W = 256              # local width (4 heads x 64)
MT = 2               # m-tiles of 128
KO8 = D // P         # 8 bf16 contraction ktiles
KO4 = D // 256       # 4 DoubleRow ktiles
NQ = S // 512        # 4 sq chunks
NPAIR = S // 256     # 8 sk chunk-pairs
HW = 65              # per-head moving width in P@V (64 dims + rowsum col)
SCALE = 0.125 / 256.0          # qt8/kt8 carry x16 each
A_SCH = 8.0 / np.log(2.0) * SCALE
B_SCH = 55.6

_CACHE = {}


# exp engine split: deterministic weighted round-robin (Bresenham).
class _ExpAssign:
    def __init__(self, wa, wd):
        self.w = {"A": wa, "D": wd}
        self.acc = {"A": 0.0, "D": 0.0}

    def next(self):
        for k in self.acc:
            self.acc[k] += self.w[k]
        k = max(self.acc, key=lambda e: self.acc[e])
        self.acc[k] -= sum(self.w.values())
        return k


def build_nc():
    nc = bacc.Bacc("TRN2", target_bir_lowering=False, debug=False, num_devices=8)
    xb = nc.dram_tensor("xb", [D, S], BF, kind="ExternalInput").ap()
    x8 = nc.dram_tensor("x8", [D, S], F8, kind="ExternalInput").ap()
    wq8 = nc.dram_tensor("wq8", [D, W], F8, kind="ExternalInput").ap()
    wk8 = nc.dram_tensor("wk8", [D, W], F8, kind="ExternalInput").ap()
    wv = nc.dram_tensor("wv", [D, W], BF, kind="ExternalInput").ap()
    wo = nc.dram_tensor("wo", [W, D], BF, kind="ExternalInput").ap()
    bq = nc.dram_tensor("bq", [P, MT], F32, kind="ExternalInput").ap()
    bk = nc.dram_tensor("bk", [P, MT], F32, kind="ExternalInput").ap()
    out = nc.dram_tensor("out", [D, S], BF, kind="ExternalOutput").ap()

    x8_r = x8.rearrange("(ko t p) s -> p ko t s", p=P, t=2)
    xb_r = xb.rearrange("(ko p) s -> p ko s", p=P)
    wq8_r = wq8.rearrange("(ko t p) w -> p ko t w", p=P, t=2)
    wk8_r = wk8.rearrange("(ko t p) w -> p ko t w", p=P, t=2)
    wv_r = wv.rearrange("(ko p) w -> p ko w", p=P)
    wo_r = wo.rearrange("(m p) d -> p m d", p=P)

    with tile.TileContext(nc) as tc, ExitStack() as ctx:
        sb = ctx.enter_context(tc.tile_pool(name="sb", bufs=1))
        xp = ctx.enter_context(tc.tile_pool(name="xp", bufs=2))
        ptp = ctx.enter_context(tc.tile_pool(name="ptp", bufs=4))
        cp = ctx.enter_context(tc.tile_pool(name="cp", bufs=8))
        ps = ctx.enter_context(tc.tile_pool(name="ps", bufs=1, space="PSUM"))

        # ---- resident inputs (issue order = DMA priority) ----
        w8q_t = sb.tile([P, KO4, 2, W], F8)
        nc.scalar.dma_start(w8q_t[:], wq8_r)
        x8_tiles = [xp.tile([P, KO4, 2, 512], F8, tag="x8", name=f"x8_{i}",
                            bufs=4) for i in range(4)]

        def load_x8(qtr):
            sq = slice(qtr * 512, (qtr + 1) * 512)
            nc.sync.dma_start(x8_tiles[qtr][:, 0:2, :, :], x8_r[:, 0:2, :, sq])
            nc.sync.dma_start(x8_tiles[qtr][:, 2:4, :, :], x8_r[:, 2:4, :, sq])

        sq0 = slice(0, 512)
        nc.sync.dma_start(x8_tiles[0][:, 0:2, :, :], x8_r[:, 0:2, :, sq0])
        bq_t = sb.tile([P, MT], F32)
        nc.sync.dma_start(bq_t[:], bq)
        nc.sync.dma_start(x8_tiles[0][:, 2:4, :, :], x8_r[:, 2:4, :, sq0])
        w8k_t = sb.tile([P, KO4, 2, W], F8)
        nc.scalar.dma_start(w8k_t[:], wk8_r)
        bk_t = sb.tile([P, MT], F32)
        nc.sync.dma_start(bk_t[:], bk)

        xq_tiles = [None] * 4

        def load_xq(qtr):
            sq = slice(qtr * 512, (qtr + 1) * 512)
            xq_tiles[qtr] = xp.tile([P, KO8, 512], BF, tag="xq", bufs=2,
                                    name=f"xq{qtr}")
            nc.sync.dma_start(xq_tiles[qtr][:, 0:KO8 // 2, :],
                              xb_r[:, 0:KO8 // 2, sq])
            nc.sync.dma_start(xq_tiles[qtr][:, KO8 // 2:KO8, :],
                              xb_r[:, KO8 // 2:KO8, sq])

        load_xq(0)
        wv_t = sb.tile([P, KO8, W], BF)
        nc.sync.dma_start(wv_t[:], wv_r)
        for _q in (1, 2, 3):
            load_x8(_q)
        load_xq(1)
        wo_t = sb.tile([P, MT, D], BF)
        nc.sync.dma_start(wo_t[:], wo_r)

        zt = sb.tile([P, 1], BF)
        nc.vector.memset(zt[:], 0.0)

        # qt8/kt8: two [64, m, S] tiles (head-pairs); within a tile head
        # h%2 sits at base 32*(h%2) (base 96 is illegal for matmul operands),
        # dk = 32m + p%32
        qt8_m = [sb.tile([64, MT, S], F8, name=f"qt8_{j}") for j in range(2)]
        kt8_m = [sb.tile([64, MT, S], F8, name=f"kt8_{j}") for j in range(2)]
        # vaug A/R: [sk-part, pair, parity, head*65]; per head 64 V-dims
        # then 1 rowsum col (A: 16.0; R: 0.0)
        vaugA_t = sb.tile([P, NPAIR, 2, 4 * HW], F8)
        vaugR_t = sb.tile([P, NPAIR, 2, 4 * HW], F8)
        consA = vaugA_t[:].rearrange("p i t (h c) -> p i t h c", c=HW)[:, :, :, :, 64:HW]
        nc.gpsimd.memset(consA, 16.0)
        consR = vaugR_t[:].rearrange("p i t (h c) -> p i t h c", c=HW)[:, :, :, :, 64:HW]
        nc.gpsimd.memset(consR, 0.0)
        ot_t = sb.tile([P, MT, S], BF)

        ea_p1 = _ExpAssign(1.0, 1.0)      # phase-1 blocks
        ea_p2 = _ExpAssign(1.05, 1.0)     # phase-2 mix

        _uid = [0]
        pending_out = []
        pending_tp = []
        pending_dma = []

        ea = _ExpAssign(1.08, 1.0)

        attnP_tiles = {}

        def get_attnP(q):
            if q not in attnP_tiles:
                attnP_tiles[q] = cp.tile([P, 4, 2, P], BF, tag="attnP",
                                         bufs=2, name=f"attnP{q}")
            return attnP_tiles[q]

        def emit_tp(q, hp, c, attnP):
            nc.sync.dma_start(
                ot_t[:, hp, q * 512 + c * 128: q * 512 + (c + 1) * 128],
                attnP[:, c, hp, :], transpose=True)
            if hp == 1 and c == 3:
                outproj(q)

        def emit_dma(q, dc0, ob):
            sq = slice(q * 512, (q + 1) * 512)
            dr = out[dc0 * P:(dc0 + 2) * P, sq].rearrange(
                "(a p) n -> p a n", a=2)
            nc.sync.dma_start(dr, ob[:])

        _ob_cur = [None]

        def outproj_unit(q, dc, ceng):
            pot = ps.tile([P, 1024], F32, tag="sp", bufs=3, name="pot")
            po = pot[:, 0:512]
            sq = slice(q * 512, (q + 1) * 512)
            for m in range(MT):
                nc.tensor.matmul(po[:], wo_t[:, m, dc * P:(dc + 1) * P],
                                 ot_t[:, m, sq],
                                 start=(m == 0), stop=(m == MT - 1))
            if dc % 2 == 0:
                _ob_cur[0] = cp.tile([P, 2, 512], BF, tag="ob", bufs=3, name="ob")
            ob = _ob_cur[0]
            half = ob[:, dc % 2, :]
            if ceng == "A":
                nc.scalar.activation(half, po[:],
                                     mybir.ActivationFunctionType.Copy,
                                     bias=0.0, scale=1.0)
            else:
                nc.vector.tensor_copy(half, po[:])
            if dc % 2 == 1:
                pending_dma.append((q, dc - 1, ob))

        def outproj(q):
            for dc in range(8):
                pending_out.append((q, dc, "A" if dc % 4 != 3 else "D"))

        def qk_proj(qtr, wt, bt, dst):
            sq = slice(qtr * 512, (qtr + 1) * 512)
            ppt = ps.tile([P, 1024], F32, tag="sp", bufs=3, name="ppt")
            for m in range(MT):
                for ko in range(KO4):
                    nc.tensor.matmul(ppt[:, m * 512:(m + 1) * 512],
                                     wt[:, ko, :, m * P:(m + 1) * P],
                                     x8_tiles[qtr][:, ko, :, :],
                                     start=(ko == 0), stop=(ko == KO4 - 1),
                                     perf_mode=DR)
            # head-pair 0 (psum parts 0:64) on DVE in one op; head-pair 1
            # (parts 64:128, partition-shifted) on ACT as two per-m ops
            nc.vector.tensor_tensor(
                dst[0][:, :, sq],
                ppt[0:64, :].rearrange("p (m n) -> p m n", n=512),
                bt[0:64, :].rearrange("p (m j) -> p m j", j=1)
                .broadcast_to([64, MT, 512]),
                mybir.AluOpType.add)
            for m in range(MT):
                nc.scalar.activation(
                    dst[1][:, m, sq], ppt[64:P, m * 512:(m + 1) * 512],
                    mybir.ActivationFunctionType.Identity,
                    bias=bt[64:P, m:m + 1], scale=1.0)

        def v_proj(qtr, st2):
            if st2 == 0:
                v_proj.pvt = ps.tile([P, 1024], F32, tag="sp", bufs=3,
                                     name="pvt")
            pvt = v_proj.pvt
            for st in (2 * st2, 2 * st2 + 1):
                pv = pvt[:, st * W:(st + 1) * W]
                for ko in range(KO8):
                    nc.tensor.matmul(pv, xq_tiles[qtr][:, ko, st * P:(st + 1) * P],
                                     wv_t[:, ko, :], start=(ko == 0),
                                     stop=(ko == KO8 - 1))

        def v_finish(qtr):
            pvt = v_proj.pvt
            vb = cp.tile([P, 1024], BF, tag="vb", bufs=2, name="vb")
            nc.scalar.activation(vb[:], pvt[:],
                                 mybir.ActivationFunctionType.Copy,
                                 bias=0.0, scale=1.0)
            vbv = vb[:].rearrange("p (a b h c) -> p a b h c", a=2, b=2, c=64)
            Av = vaugA_t[:, 2 * qtr:2 * qtr + 2, :, :].rearrange(
                "p a b (h c) -> p a b h c", c=HW)[:, :, :, :, 0:64]
            Rv = vaugR_t[:, 2 * qtr:2 * qtr + 2, :, :].rearrange(
                "p a b (h c) -> p a b h c", c=HW)[:, :, :, :, 0:64]
            nc.gpsimd.tensor_tensor(Av, vbv,
                                    zt[:].rearrange("p (a b h c) -> p a b h c",
                                                    a=1, b=1, h=1)
                                    .broadcast_to([P, 2, 2, 4, 64]),
                                    mybir.AluOpType.add)
            nc.gpsimd.tensor_tensor(Rv, vbv, Av, mybir.AluOpType.subtract)

        class Block:
            def __init__(self, q, h):
                _uid[0] += 1
                self.u = _uid[0]
                self.q, self.h = q, h
                self.sq = slice(q * 512, (q + 1) * 512)
                self.hq = slice(32 * (h % 2), 32 * (h % 2) + 32)
                self.qt8, self.kt8 = qt8_m[h // 2], kt8_m[h // 2]
                self.hc = slice(h * HW, (h + 1) * HW)
                self.pairs = 0          # score/exp steps emitted
                self.pts = [None] * NPAIR
                self.pv_done = 0
                self.oPf = None
                self.done = False

            def can_score(self, qtr):
                return (self.pairs < NPAIR and self.q <= qtr
                        and self.pairs // 2 <= qtr)

            def step_score(self):
                i = self.pairs
                pt = ptp.tile([P, 2, 512], F8, tag="pt", bufs=36,
                              name=f"pt{self.u}_{i}")
                sp = ps.tile([P, 1024], F32, tag="sp", bufs=3,
                             name=f"sp{self.u}_{i}")
                for half in (0, 1):
                    ks = slice((2 * i + half) * P, (2 * i + half + 1) * P)
                    nc.tensor.matmul(sp[:, half * 512:(half + 1) * 512],
                                     self.kt8[self.hq, :, ks],
                                     self.qt8[self.hq, :, self.sq],
                                     start=True, stop=True, perf_mode=DR)
                ptf = pt[:].rearrange("p t n -> p (t n)")
                if ea.next() == "A":
                    nc.scalar.activation(ptf, sp[:],
                                         mybir.ActivationFunctionType.Exp,
                                         bias=0.0, scale=SCALE)
                else:
                    nc.vector.tensor_scalar(ptf.bitcast(I8), sp[:],
                                            A_SCH, B_SCH,
                                            mybir.AluOpType.mult,
                                            mybir.AluOpType.add)
                self.pts[i] = pt
                self.pairs += 1

            def grant(self):
                self.oPf = ps.tile([P, 512], F32, tag="oPf", bufs=2,
                                   name=f"oPf{self.u}")

            def pv_ready(self, vfin_qtr):
                # PV pair i needs exp lag 2 and vaug through qtr i//2
                i = self.pv_done
                return (self.oPf is not None and i < NPAIR
                        and (i <= self.pairs - 3 or self.pairs == NPAIR)
                        and i // 2 <= vfin_qtr)

            def step_pv(self):
                i = self.pv_done
                oP = self.oPf[:].rearrange("p (c n) -> p c n", n=128)[:, :, 0:HW]
                pt = self.pts[i]
                for c in range(4):
                    stat = pt[:, :, c * 128:(c + 1) * 128]
                    nc.tensor.matmul(oP[:, c, :], stat,
                                     vaugA_t[:, i, :, self.hc],
                                     start=(i == 0), stop=False,
                                     perf_mode=DR)
                    nc.tensor.matmul(oP[:, c, :], stat,
                                     vaugR_t[:, i, :, self.hc],
                                     start=False, stop=(i == NPAIR - 1),
                                     perf_mode=DR)
                self.pts[i] = None
                self.pv_done = i + 1

            def epilogue(self):
                oP = self.oPf[:].rearrange("p (c n) -> p c n", n=128)[:, :, 0:HW]
                attnP = get_attnP(self.q)
                h = self.h
                rs = cp.tile([P, 4], F32, tag="rs", bufs=4, name=f"rs{self.u}")
                nc.vector.reciprocal(
                    rs[:], oP[:, :, 64:HW].rearrange("p c j -> p (c j)"))
                nc.vector.tensor_tensor(
                    attnP[:, :, h // 2, 64 * (h % 2):64 * (h % 2) + 64],
                    oP[:, :, 0:64],
                    rs[:].rearrange("p (c j) -> p c j", j=1)
                    .broadcast_to([P, 4, 64]),
                    mybir.AluOpType.mult)
                self.done = True
                self.oPf = None
                if h % 2 == 1:
                    pending_tp.extend(
                        (self.q, h // 2, c, attnP) for c in range(4))

        # ---------------- unified schedule ----------------
        proj_items = []
        for t in range(NQ):
            items = [("qkq", t), ("qkk", t),
                     ("v0", t), ("v1", t), ("vfin", t)]
            if t + 1 < NQ:
                items.insert(2, ("ldx", t + 1))
            proj_items.append(items)

        def emit_proj(kind, t):
            if kind == "ldx":
                if t > 1:
                    load_xq(t)
                if t == 1:
                    pass
            elif kind == "qkq":
                qk_proj(t, w8q_t, bq_t, qt8_m)
            elif kind == "qkk":
                qk_proj(t, w8k_t, bk_t, kt8_m)
            elif kind == "v0":
                v_proj(t, 0)
            elif kind == "v1":
                v_proj(t, 1)
            elif kind == "vfin":
                v_finish(t)
                vfin_done[0] = t

        vfin_done = [-1]
        all_blocks = [Block(q, h) for q in range(NQ) for h in range(4)]
        unstarted = list(all_blocks)
        active = []
        MAXA = 5
        SLOTS = 2
        slots_used = [0]
        qtr = [-1]
        pq = []   # flat pending proj emissions

        def advance_qtr():
            if qtr[0] < NQ - 1:
                qtr[0] += 1
                pq.extend(proj_items[qtr[0]])
                return True
            return False

        advance_qtr()
        while pq:
            k, t = pq.pop(0)
            emit_proj(k, t)

        def refill():
            while (len(active) < MAXA and unstarted
                   and unstarted[0].q <= qtr[0]):
                active.append(unstarted.pop(0))

        def pump_pv(cap=1):
            n = 0
            for b in active:
                while (b.oPf is not None and b.pv_ready(vfin_done[0])
                       and (cap is None or n < cap)):
                    b.step_pv()
                    n += 1
                if (b.oPf is not None and b.pv_done == NPAIR
                        and not b.done):
                    b.epilogue()
                    slots_used[0] -= 1
            while [b for b in active if b.done]:
                active.remove([b for b in active if b.done][0])

        def grant_slots():
            for b in active:
                if slots_used[0] >= SLOTS:
                    break
                if b.oPf is None and not b.done and b.pairs > 0:
                    b.grant()
                    slots_used[0] += 1

        tick = [0]
        while True:
            refill()
            grant_slots()
            outstanding = sum(b.pairs - b.pv_done for b in active)
            cand = None
            if outstanding < 24:
                cand = next((b for b in active if b.can_score(qtr[0])), None)
            elif any(b.oPf is not None and b.pv_ready(vfin_done[0])
                     for b in active):
                pass  # let pv drain below
            else:
                cand = next((b for b in active if b.can_score(qtr[0])), None)
            if cand is None:
                if pq:
                    k, t = pq.pop(0)
                    emit_proj(k, t)
                    continue
                if advance_qtr():
                    continue
                if not active and not unstarted:
                    break
                # blocks waiting on pv/epilogue only
                pump_pv(cap=None)
                grant_slots()
                if pending_tp:
                    emit_tp(*pending_tp.pop(0))
                if pending_out:
                    outproj_unit(*pending_out.pop(0))
                if pending_dma:
                    emit_dma(*pending_dma.pop(0))
                continue
            cand.step_score()
            tick[0] += 1
            pump_pv()
            if pending_tp:
                emit_tp(*pending_tp.pop(0))
            elif pending_dma:
                emit_dma(*pending_dma.pop(0))
            if pending_out and (tick[0] % 2 == 0 or len(pending_out) > 2):
                outproj_unit(*pending_out.pop(0))
        while pending_tp:
            emit_tp(*pending_tp.pop(0))
        while pending_out:
            q, dc, ceng = pending_out.pop(0)
            outproj_unit(q, dc, "D" if len(pending_out) == 0 else ceng)
        while pending_dma:
            emit_dma(*pending_dma.pop(0))
    nc.compile()
    return nc


def _prep_inputs(x, Wq, bq, Wk, bk, Wv, bv, Wo, bo):
    # straight perm: psum partition p of m-tile m holds W-col
    # (p//32)*64 + 32*m + (p%32)  (head p//32, dk-dim 32m + p%32)
    perm = np.empty(W, dtype=np.int64)
    for m in range(MT):
        p = np.arange(P)
        perm[m * P + p] = (p // 32) * 64 + 32 * m + (p % 32)

    in_maps = []
    for c in range(8):
        b, g = c // 4, c % 4
        cs = slice(g * W, (g + 1) * W)
        xTb = np.ascontiguousarray(x[b].T)
        Wq_l, bq_l = Wq[:, cs][:, perm], bq[cs][perm]
        Wk_l, bk_l = Wk[:, cs][:, perm], bk[cs][perm]
        in_maps.append({
            "xb": xTb.astype(BF16),
            "x8": xTb.astype(E4),
            "wq8": (16.0 * Wq_l).astype(E4),
            "wk8": (16.0 * Wk_l).astype(E4),
            "wv": (16.0 * Wv[:, cs]).astype(BF16),
            "wo": Wo[cs, :].astype(BF16),
            "bq": np.ascontiguousarray(16.0 * bq_l.reshape(MT, P).T),
            "bk": np.ascontiguousarray(16.0 * bk_l.reshape(MT, P).T),
        })
    return in_maps


def kernel(x, Wq, bq, Wk, bk, Wv, bv, Wo, bo):
    x = np.asarray(x, dtype=np.float32)
    Wq, bq = np.asarray(Wq, np.float32), np.asarray(bq, np.float32)
    Wk, bk = np.asarray(Wk, np.float32), np.asarray(bk, np.float32)
    Wv, bv = np.asarray(Wv, np.float32), np.asarray(bv, np.float32)
    Wo, bo = np.asarray(Wo, np.float32), np.asarray(bo, np.float32)

    if "nc" not in _CACHE:
        _CACHE["nc"] = build_nc()
    nc = _CACHE["nc"]

    in_maps = _prep_inputs(x, Wq, bq, Wk, bk, Wv, bv, Wo, bo)
    res = run_bass_kernel_spmd(nc, in_maps, core_ids=list(range(8))).results

    extra = bv @ Wo + bo   # bv folded out of the V projection
    out = np.empty((B, S, D), dtype=np.float32)
    for b in range(B):
        acc = res[4 * b]["out"].astype(np.float32)
        for g in range(1, 4):
            acc += res[4 * b + g]["out"].astype(np.float32)
        out[b] = acc.T + extra
    return out


# revision 17
# speedup vs baseline: 1.2360x; 1.0255x over previous
"""Multi-head attention (B=2, S=2048, D=1024, H=16, Dk=64) on 8 NeuronCores.

Sharding: 2-way data parallel over batch x 4-way tensor parallel over heads.
Core c = 4*b + g handles batch b, head group g (4 heads = 256 cols).
W_o is row-sliced; host sums the 4 partial outputs per batch (+ bv@Wo + bo).

v3 (elementwise-bound rework; ACT/DVE are the binding engines):
  - Q/K projections: fp8 DoubleRow as before, but the W-column permutation is
    chosen so each m-tile's psum [128,512] maps partition-straight into a
    single [128,2,S] qt8/kt8 tile (head h at partition base 32h, legal
    quadrant bases). The psum->sbuf move is ONE full-partition op per
    projection: DVE tensor_tensor add with a broadcast per-(p,m) bias tile;
    no /16 rescale (scales folded into the exp constant).
  - scores: per-head DR with dk=64 at quadrant base 32h (unchanged math).
  - softmax exp: ACT exact Exp and DVE Schraudolph split by weighted
    round-robin, scale = 0.125/256.
  - P@V FLIPPED: stationary = probs chunk [128,2,128], moving = vaug
    [128,2,65] (64 V-dims + 1 rowsum const col per head); out psum
    [128 tok, 4, 65]. Full A+R residual compensation (V stored as fp8 A
    plus equal-scale fp8 residual R). 65-wide outputs cut P@V PE time by
    ~35% and shrink the normalization to [128,4,64] broadcast ops.
  - normalization: DVE reciprocal [128,4] + one broadcast tensor_tensor mul
    writing bf16 attnP; DMA-transpose ([128,128] bf16 tiles, 14ns/xbar-tile)
    rebuilds the [dims, tok] layout for the output projection.
  - V path: V-proj psum -> ONE bf16 copy per qtr; Pool (gpsimd) builds fp8
    A and R from SBUF via tensor_tensor (GPSIMD cannot touch PSUM).
  - output projection: flipped (out^T [D,S] in DRAM, host transposes),
    moving = ot_t [128,2,512] bf16, stationary = Wo; psum->sbuf copies on
    ACT. bq/bk biases on-chip; bv/bo folded into the host-side reduction.
"""
import numpy as np
import ml_dtypes
from contextlib import ExitStack

import concourse.bass as bass
import concourse.mybir as mybir
import concourse.tile as tile
from concourse import bacc
from concourse.bass_utils import run_bass_kernel_spmd

F32 = mybir.dt.float32
F8 = mybir.dt.float8e4
I8 = mybir.dt.int8
BF = mybir.dt.bfloat16
E4 = ml_dtypes.float8_e4m3
BF16 = ml_dtypes.bfloat16
DR = mybir.MatmulPerfMode.DoubleRow

B, S, D = 2, 2048, 1024
P = 128
W = 256              # local width (4 heads x 64)
MT = 2               # m-tiles of 128
KO8 = D // P         # 8 bf16 contraction ktiles
KO4 = D // 256       # 4 DoubleRow ktiles
NQ = S // 512        # 4 sq chunks
NPAIR = S // 256     # 8 sk chunk-pairs
HW = 65              # per-head moving width in P@V (64 dims + rowsum col)
SCALE = 0.125 / 256.0          # qt8/kt8 carry x16 each
A_SCH = 8.0 / np.log(2.0) * SCALE
B_SCH = 55.6

_CACHE = {}


# exp engine split: deterministic weighted round-robin (Bresenham).
class _ExpAssign:
    def __init__(self, wa, wd):
        self.w = {"A": wa, "D": wd}
        self.acc = {"A": 0.0, "D": 0.0}

    def next(self):
        for k in self.acc:
            self.acc[k] += self.w[k]
        k = max(self.acc, key=lambda e: self.acc[e])
        self.acc[k] -= sum(self.w.values())
        return k


def build_nc():
    nc = bacc.Bacc("TRN2", target_bir_lowering=False, debug=False, num_devices=8)
    xb = nc.dram_tensor("xb", [D, S], BF, kind="ExternalInput").ap()
    x8 = nc.dram_tensor("x8", [D, S], F8, kind="ExternalInput").ap()
    wq8 = nc.dram_tensor("wq8", [D, W], F8, kind="ExternalInput").ap()
    wk8 = nc.dram_tensor("wk8", [D, W], F8, kind="ExternalInput").ap()
    wv = nc.dram_tensor("wv", [D, W], BF, kind="ExternalInput").ap()
    wo = nc.dram_tensor("wo", [W, D], BF, kind="ExternalInput").ap()
    bq = nc.dram_tensor("bq", [P, MT], F32, kind="ExternalInput").ap()
    bk = nc.dram_tensor("bk", [P, MT], F32, kind="ExternalInput").ap()
    out = nc.dram_tensor("out", [D, S], BF, kind="ExternalOutput").ap()

    x8_r = x8.rearrange("(ko t p) s -> p ko t s", p=P, t=2)
    xb_r = xb.rearrange("(ko p) s -> p ko s", p=P)
    wq8_r = wq8.rearrange("(ko t p) w -> p ko t w", p=P, t=2)
    wk8_r = wk8.rearrange("(ko t p) w -> p ko t w", p=P, t=2)
    wv_r = wv.rearrange("(ko p) w -> p ko w", p=P)
    wo_r = wo.rearrange("(m p) d -> p m d", p=P)

    with tile.TileContext(nc) as tc, ExitStack() as ctx:
        sb = ctx.enter_context(tc.tile_pool(name="sb", bufs=1))
        xp = ctx.enter_context(tc.tile_pool(name="xp", bufs=2))
        ptp = ctx.enter_context(tc.tile_pool(name="ptp", bufs=4))
        cp = ctx.enter_context(tc.tile_pool(name="cp", bufs=8))
        ps = ctx.enter_context(tc.tile_pool(name="ps", bufs=1, space="PSUM"))

        # ---- resident inputs (issue order = DMA priority) ----
        w8q_t = sb.tile([P, KO4, 2, W], F8)
        nc.scalar.dma_start(w8q_t[:], wq8_r)
        x8_tiles = [xp.tile([P, KO4, 2, 512], F8, tag="x8", name=f"x8_{i}",
                            bufs=4) for i in range(4)]

        def load_x8(qtr):
            sq = slice(qtr * 512, (qtr + 1) * 512)
            nc.sync.dma_start(x8_tiles[qtr][:, 0:2, :, :], x8_r[:, 0:2, :, sq])
            nc.sync.dma_start(x8_tiles[qtr][:, 2:4, :, :], x8_r[:, 2:4, :, sq])

        sq0 = slice(0, 512)
        nc.sync.dma_start(x8_tiles[0][:, 0:2, :, :], x8_r[:, 0:2, :, sq0])
        bq_t = sb.tile([P, MT], F32)
        nc.sync.dma_start(bq_t[:], bq)
        nc.sync.dma_start(x8_tiles[0][:, 2:4, :, :], x8_r[:, 2:4, :, sq0])
        w8k_t = sb.tile([P, KO4, 2, W], F8)
        nc.scalar.dma_start(w8k_t[:], wk8_r)
        bk_t = sb.tile([P, MT], F32)
        nc.sync.dma_start(bk_t[:], bk)

        xq_tiles = [None] * 4

        def load_xq(qtr):
            sq = slice(qtr * 512, (qtr + 1) * 512)
            xq_tiles[qtr] = xp.tile([P, KO8, 512], BF, tag="xq", bufs=2,
                                    name=f"xq{qtr}")
            nc.sync.dma_start(xq_tiles[qtr][:, 0:KO8 // 2, :],
                              xb_r[:, 0:KO8 // 2, sq])
            nc.sync.dma_start(xq_tiles[qtr][:, KO8 // 2:KO8, :],
                              xb_r[:, KO8 // 2:KO8, sq])

        load_xq(0)
        wv_t = sb.tile([P, KO8, W], BF)
        nc.sync.dma_start(wv_t[:], wv_r)
        for _q in (1, 2, 3):
            load_x8(_q)
        load_xq(1)
        wo_t = sb.tile([P, MT, D], BF)
        nc.sync.dma_start(wo_t[:], wo_r)

        zt = sb.tile([P, 1], BF)
        nc.vector.memset(zt[:], 0.0)

        # qt8/kt8: two [64, m, S] tiles (head-pairs); within a tile head
        # h%2 sits at base 32*(h%2) (base 96 is illegal for matmul operands),
        # dk = 32m + p%32
        qt8_m = [sb.tile([64, MT, S], F8, name=f"qt8_{j}") for j in range(2)]
        kt8_m = [sb.tile([64, MT, S], F8, name=f"kt8_{j}") for j in range(2)]
        # vaug A/R: [sk-part, pair, parity, head*65]; per head 64 V-dims
        # then 1 rowsum col (A: 16.0; R: 0.0)
        vaugA_t = sb.tile([P, NPAIR, 2, 4 * HW], F8)
        vaugR_t = sb.tile([P, NPAIR, 2, 4 * HW], F8)
        consA = vaugA_t[:].rearrange("p i t (h c) -> p i t h c", c=HW)[:, :, :, :, 64:HW]
        nc.gpsimd.memset(consA, 16.0)
        consR = vaugR_t[:].rearrange("p i t (h c) -> p i t h c", c=HW)[:, :, :, :, 64:HW]
        nc.gpsimd.memset(consR, 0.0)
        ot_t = sb.tile([P, MT, S], BF)

        ea_p1 = _ExpAssign(1.0, 1.0)      # phase-1 blocks
        ea_p2 = _ExpAssign(1.05, 1.0)     # phase-2 mix

        _uid = [0]
        pending_out = []
        pending_tp = []
        pending_dma = []

        # attnP per q-block: [tok-part, chunk, head-pair, 128 dims] bf16
        def attn_block(q, h, ea, attnP):
            _uid[0] += 1
            u = _uid[0]
            sq = slice(q * 512, (q + 1) * 512)
            hq = slice(32 * (h % 2), 32 * (h % 2) + 32)
            qt8, kt8 = qt8_m[h // 2], kt8_m[h // 2]
            hc = slice(h * HW, (h + 1) * HW)
            oPt = ps.tile([P, 512], F32, tag="oP", bufs=2, name=f"oP{u}")
            oP = oPt[:].rearrange("p (c n) -> p c n", n=128)[:, :, 0:HW]

            def attn_v(i, pt):
                for c in range(4):
                    stat = pt[:, :, c * 128:(c + 1) * 128]
                    nc.tensor.matmul(oP[:, c, :], stat, vaugA_t[:, i, :, hc],
                                     start=(i == 0), stop=False,
                                     perf_mode=DR)
                    nc.tensor.matmul(oP[:, c, :], stat, vaugR_t[:, i, :, hc],
                                     start=False, stop=(i == NPAIR - 1),
                                     perf_mode=DR)

            pend_av = []
            for i in range(NPAIR):
                pt = ptp.tile([P, 2, 512], F8, tag="pt", bufs=10,
                              name=f"pt{u}_{i}")
                sp = ps.tile([P, 1024], F32, tag="sp", bufs=3,
                             name=f"sp{u}_{i}")
                for half in (0, 1):
                    ks = slice((2 * i + half) * P, (2 * i + half + 1) * P)
                    nc.tensor.matmul(sp[:, half * 512:(half + 1) * 512],
                                     kt8[hq, :, ks], qt8[hq, :, sq],
                                     start=True, stop=True, perf_mode=DR)
                ptf = pt[:].rearrange("p t n -> p (t n)")
                eng = ea.next()
                if eng == "A":
                    nc.scalar.activation(ptf, sp[:],
                                         mybir.ActivationFunctionType.Exp,
                                         bias=0.0, scale=SCALE)
                else:
                    nc.vector.tensor_scalar(ptf.bitcast(I8), sp[:],
                                            A_SCH, B_SCH,
                                            mybir.AluOpType.mult,
                                            mybir.AluOpType.add)
                if len(pend_av) >= 2:
                    attn_v(*pend_av.pop(0))
                pend_av.append((i, pt))
                if pending_tp:
                    emit_tp(*pending_tp.pop(0))
                elif pending_dma:
                    emit_dma(*pending_dma.pop(0))
                if pending_out and i >= 4 and (i % 2 == 0
                                               or len(pending_out) >= 6):
                    outproj_unit(*pending_out.pop(0))
                yield i
            while pend_av:
                attn_v(*pend_av.pop(0))
            # normalization: rs = 1/rowsum; attnP slice = oP[:, :, 0:64]*rs
            rs = cp.tile([P, 4], F32, tag="rs", bufs=4, name=f"rs{u}")
            nc.vector.reciprocal(rs[:], oP[:, :, 64:HW].rearrange("p c j -> p (c j)"))
            nc.vector.tensor_tensor(
                attnP[:, :, h // 2, 64 * (h % 2):64 * (h % 2) + 64],
                oP[:, :, 0:64],
                rs[:].rearrange("p (c j) -> p c j", j=1).broadcast_to([P, 4, 64]),
                mybir.AluOpType.mult)
            yield NPAIR

        def emit_tp(q, hp, c, attnP):
            nc.sync.dma_start(
                ot_t[:, hp, q * 512 + c * 128: q * 512 + (c + 1) * 128],
                attnP[:, c, hp, :], transpose=True)
            if hp == 1 and c == 3:
                outproj(q)

        def transposes(q, hp, attnP):
            pending_tp.extend((q, hp, c, attnP) for c in range(4))

        def qk_proj(qtr, wt, bt, dst):
            sq = slice(qtr * 512, (qtr + 1) * 512)
            ppt = ps.tile([P, 1024], F32, tag="sp", bufs=3)
            for m in range(MT):
                for ko in range(KO4):
                    nc.tensor.matmul(ppt[:, m * 512:(m + 1) * 512],
                                     wt[:, ko, :, m * P:(m + 1) * P],
                                     x8_tiles[qtr][:, ko, :, :],
                                     start=(ko == 0), stop=(ko == KO4 - 1),
                                     perf_mode=DR)
            # head-pair 0 (psum parts 0:64) on DVE in one op; head-pair 1
            # (parts 64:128, partition-shifted) on ACT as two per-m ops
            nc.vector.tensor_tensor(
                dst[0][:, :, sq],
                ppt[0:64, :].rearrange("p (m n) -> p m n", n=512),
                bt[0:64, :].rearrange("p (m j) -> p m j", j=1)
                .broadcast_to([64, MT, 512]),
                mybir.AluOpType.add)
            for m in range(MT):
                nc.scalar.activation(
                    dst[1][:, m, sq], ppt[64:P, m * 512:(m + 1) * 512],
                    mybir.ActivationFunctionType.Identity,
                    bias=bt[64:P, m:m + 1], scale=1.0)

        def v_proj_mm(qtr, st2):
            # two st chunks of matmuls into the shared pvt tile
            if st2 == 0:
                pvt = ps.tile([P, 1024], F32, tag="sp", bufs=3)
                v_proj_mm.pvt = pvt
            else:
                pvt = v_proj_mm.pvt
            for st in (2 * st2, 2 * st2 + 1):
                pv = pvt[:, st * W:(st + 1) * W]
                for ko in range(KO8):
                    nc.tensor.matmul(pv, xq_tiles[qtr][:, ko, st * P:(st + 1) * P],
                                     wv_t[:, ko, :], start=(ko == 0),
                                     stop=(ko == KO8 - 1))
            return pvt

        def v_finish(qtr, pvt):
            vb = cp.tile([P, 1024], BF, tag="vb", bufs=2)
            nc.scalar.activation(vb[:], pvt[:],
                                 mybir.ActivationFunctionType.Copy,
                                 bias=0.0, scale=1.0)
            # Pool: A = fp8(vb), R = fp8(vb - A); layout [p, st(a b), h, 64]
            vbv = vb[:].rearrange("p (a b h c) -> p a b h c", a=2, b=2, c=64)
            Av = vaugA_t[:, 2 * qtr:2 * qtr + 2, :, :].rearrange(
                "p a b (h c) -> p a b h c", c=HW)[:, :, :, :, 0:64]
            Rv = vaugR_t[:, 2 * qtr:2 * qtr + 2, :, :].rearrange(
                "p a b (h c) -> p a b h c", c=HW)[:, :, :, :, 0:64]
            nc.gpsimd.tensor_tensor(Av, vbv,
                                    zt[:].rearrange("p (a b h c) -> p a b h c",
                                                    a=1, b=1, h=1)
                                    .broadcast_to([P, 2, 2, 4, 64]),
                                    mybir.AluOpType.add)
            nc.gpsimd.tensor_tensor(Rv, vbv, Av, mybir.AluOpType.subtract)

        _ob_cur = [None]

        def emit_dma(q, dc0, ob):
            sq = slice(q * 512, (q + 1) * 512)
            dr = out[dc0 * P:(dc0 + 2) * P, sq].rearrange(
                "(a p) n -> p a n", a=2)
            nc.sync.dma_start(dr, ob[:])

        def outproj_unit(q, dc, ceng):
            pot = ps.tile([P, 1024], F32, tag="sp", bufs=3, name="pot")
            po = pot[:, 0:512]
            sq = slice(q * 512, (q + 1) * 512)
            for m in range(MT):
                nc.tensor.matmul(po[:], wo_t[:, m, dc * P:(dc + 1) * P],
                                 ot_t[:, m, sq],
                                 start=(m == 0), stop=(m == MT - 1))
            if dc % 2 == 0:
                _ob_cur[0] = cp.tile([P, 2, 512], BF, tag="ob", bufs=3, name="ob")
            ob = _ob_cur[0]
            half = ob[:, dc % 2, :]
            if ceng == "A":
                nc.scalar.activation(half, po[:],
                                     mybir.ActivationFunctionType.Copy,
                                     bias=0.0, scale=1.0)
            else:
                nc.vector.tensor_copy(half, po[:])
            if dc % 2 == 1:
                pending_dma.append((q, dc - 1, ob))

        def outproj(q):
            for dc in range(8):
                pending_out.append((q, dc, "A" if dc % 4 != 3 else "D"))

        attnP_tiles = {}

        def get_attnP(q):
            if q not in attnP_tiles:
                attnP_tiles[q] = cp.tile([P, 4, 2, P], BF, tag="attnP",
                                         bufs=2, name=f"attnP{q}")
            return attnP_tiles[q]

        # ---- phase 1: projections with 2 interleaved attention blocks ----
        blk00 = attn_block(0, 0, ea_p1, get_attnP(0))
        blk01 = attn_block(0, 1, ea_p1, get_attnP(0))
        for qtr in range(NQ):
            if qtr > 1:
                load_xq(qtr)
            qk_proj(qtr, w8q_t, bq_t, qt8_m)
            next(blk00, None)
            qk_proj(qtr, w8k_t, bk_t, kt8_m)
            next(blk01, None)
            pvt = v_proj_mm(qtr, 0)
            next(blk00, None)
            v_proj_mm(qtr, 1)
            next(blk01, None)
            v_finish(qtr, pvt)
        next(blk00, None)   # epilogues
        next(blk01, None)
        for _ in blk00:
            pass
        for _ in blk01:
            pass
        transposes(0, 0, get_attnP(0))

        # ---- phase 2: remaining blocks, software-pipelined 2 deep ----
        blocks = [(q, h) for q in range(NQ) for h in range(4)
                  if not (q == 0 and h < 2)]
        prev_g, prev_qh = None, None
        for bi, (q, h) in enumerate(blocks):
            g = attn_block(q, h, ea_p2, get_attnP(q))
            if prev_g is None:
                for _ in range(4):
                    next(g, None)
            else:
                for _ in range(4):
                    next(prev_g, None)
                    next(g, None)
                for _ in prev_g:
                    pass
                pq, ph = prev_qh
                if ph % 2 == 1:
                    transposes(pq, ph // 2, get_attnP(pq))
            prev_g, prev_qh = g, (q, h)
        for _ in prev_g:
            pass
        transposes(NQ - 1, 1, get_attnP(NQ - 1))
        while pending_tp:
            emit_tp(*pending_tp.pop(0))
        while pending_out:
            q, dc, ceng = pending_out.pop(0)
            outproj_unit(q, dc, "D" if len(pending_out) == 0 else ceng)
        while pending_dma:
            emit_dma(*pending_dma.pop(0))
    nc.compile()
    return nc


def _prep_inputs(x, Wq, bq, Wk, bk, Wv, bv, Wo, bo):
    # straight perm: psum partition p of m-tile m holds W-col
    # (p//32)*64 + 32*m + (p%32)  (head p//32, dk-dim 32m + p%32)
    perm = np.empty(W, dtype=np.int64)
    for m in range(MT):
        p = np.arange(P)
        perm[m * P + p] = (p // 32) * 64 + 32 * m + (p % 32)

    in_maps = []
    for c in range(8):
        b, g = c // 4, c % 4
        cs = slice(g * W, (g + 1) * W)
        xTb = np.ascontiguousarray(x[b].T)
        Wq_l, bq_l = Wq[:, cs][:, perm], bq[cs][perm]
        Wk_l, bk_l = Wk[:, cs][:, perm], bk[cs][perm]
        in_maps.append({
            "xb": xTb.astype(BF16),
            "x8": xTb.astype(E4),
            "wq8": (16.0 * Wq_l).astype(E4),
            "wk8": (16.0 * Wk_l).astype(E4),
            "wv": (16.0 * Wv[:, cs]).astype(BF16),
            "wo": Wo[cs, :].astype(BF16),
            "bq": np.ascontiguousarray(16.0 * bq_l.reshape(MT, P).T),
            "bk": np.ascontiguousarray(16.0 * bk_l.reshape(MT, P).T),
        })
    return in_maps


def kernel(x, Wq, bq, Wk, bk, Wv, bv, Wo, bo):
    x = np.asarray(x, dtype=np.float32)
    Wq, bq = np.asarray(Wq, np.float32), np.asarray(bq, np.float32)
    Wk, bk = np.asarray(Wk, np.float32), np.asarray(bk, np.float32)
    Wv, bv = np.asarray(Wv, np.float32), np.asarray(bv, np.float32)
    Wo, bo = np.asarray(Wo, np.float32), np.asarray(bo, np.float32)

    if "nc" not in _CACHE:
        _CACHE["nc"] = build_nc()
    nc = _CACHE["nc"]

    in_maps = _prep_inputs(x, Wq, bq, Wk, bk, Wv, bv, Wo, bo)
    res = run_bass_kernel_spmd(nc, in_maps, core_ids=list(range(8))).results

    extra = bv @ Wo + bo   # bv folded out of the V projection
    out = np.empty((B, S, D), dtype=np.float32)
    for b in range(B):
        acc = res[4 * b]["out"].astype(np.float32)
        for g in range(1, 4):
            acc += res[4 * b + g]["out"].astype(np.float32)
        out[b] = acc.T + extra
    return out
